# revision 19
# baseline (speedup 1.0000x reference)
"""Trainium2 Bass kernel for DiffuserAttention (GNN message passing).

v2 sharding: 8-way by dst-node range (1024 dst rows per core, full 768
feature width).  Edge scores are folded into diffusion step 1 via a
concatenated [k | v] gather table, so per-edge k rows ride the step-1
message gather for free.  Edge softmax is unnormalized (escale =
0.9*exp(score)); normalization (1/denom) is applied per-dst after each
segment-sum.  Per step: dma_gather h[src] rows (1536B/edge), DVE
broadcast-scale by escale, PE one-hot segment-sum into PSUM, per-dst
rdenom scale + 0.1*v teleport, 8-rank AllGather of the h shard (steps
1-4 only; step 5 output stays local for the output projection +
residual + LayerNorm).
"""

import sys

sys.path.insert(0, "/opt/trn_rl_repo")

import math

import numpy as np
import ml_dtypes

import concourse.bass as bass
import concourse.bacc as bacc
import concourse.mybir as mybir
import concourse.tile as tile
from concourse.bass_utils import run_bass_kernel_spmd

F32 = mybir.dt.float32
BF16 = mybir.dt.bfloat16
I16 = mybir.dt.int16
NPBF16 = ml_dtypes.bfloat16

NCORES = 8
NRANGE = 8
ALPHA = 0.1
NSTEPS = 5
LN_EPS = 1e-12
GCH = 8      # chunks per gather group (1024 idx = SWDGE ring capacity)


def _cfg(B, S, D, H, E):
    N = B * S
    cfg = dict(
        B=B, S=S, D=D, H=H, E=E, N=N,
        HD=D // H,
        NR=N // NRANGE,
    )
    cfg["NBLK"] = cfg["NR"] // 128
    cfg["DC"] = D // 128
    return cfg


def wrap_idx(idx):
    """dma_gather index layout: [128, n/16] int16; idx i at [i%16, i//16],
    replicated across the 8 Q7 cores."""
    n = idx.shape[0]
    w = idx.reshape(n // 16, 16).T.astype(np.int16)
    return np.ascontiguousarray(np.tile(w, (8, 1)))


def host_prep(cfg, hidden_states, attention_mask, src, dst,
              Wq, bq, Wk, bk, Wv, bv, Wo, bo, ln_g, ln_b):
    N, D, H, HD = cfg["N"], cfg["D"], cfg["H"], cfg["HD"]
    NR, NBLK = cfg["NR"], cfg["NBLK"]

    x = np.asarray(hidden_states, np.float32).reshape(N, D)
    src = np.asarray(src).astype(np.int64)
    dst = np.asarray(dst).astype(np.int64)
    mask1 = np.asarray(attention_mask).reshape(-1) >= 0
    all_valid = bool(mask1.all())

    # ---- edge partition by dst range, sort by dst, pad per 128-dst block
    per_range = []
    maxchunks = 0
    for r in range(NRANGE):
        sel = np.nonzero((dst >= r * NR) & (dst < (r + 1) * NR))[0]
        dl = dst[sel] - r * NR
        order = np.argsort(dl, kind="stable")
        sel = sel[order]
        dl = dl[order]
        counts = np.bincount(dl >> 7, minlength=NBLK)
        maxchunks = max(maxchunks, int(np.ceil(counts / 128).max()))
        per_range.append((sel, dl, counts))

    C_BLK = maxchunks
    while (NBLK * C_BLK) % GCH:
        C_BLK += 1
    NCHUNK = NBLK * C_BLK
    EP = NCHUNK * 128

    edges = []
    for r in range(NRANGE):
        sel, dl, counts = per_range[r]
        src_e = np.zeros(EP, np.int16)
        dstloc_e = np.zeros(EP, np.float32)
        valid_e = np.zeros(EP, np.float32)
        starts = np.concatenate([[0], np.cumsum(counts)])
        for b in range(NBLK):
            s0, s1 = starts[b], starts[b + 1]
            n = s1 - s0
            o = b * C_BLK * 128
            src_e[o:o + n] = src[sel[s0:s1]]
            dstloc_e[o:o + n] = (dl[s0:s1] - b * 128).astype(np.float32)
            if all_valid:
                valid_e[o:o + n] = 1.0
            else:
                valid_e[o:o + n] = (
                    mask1[src[sel[s0:s1]]] & mask1[dst[sel[s0:s1]]]
                ).astype(np.float32)
        edges.append(dict(
            src16=wrap_idx(src_e),
            dstloc=np.ascontiguousarray(dstloc_e.reshape(NCHUNK, 128).T),
            valid=np.ascontiguousarray(
                valid_e.reshape(NCHUNK, 128).T.astype(NPBF16)),
        ))

    # ---- weights / constants
    xT = np.ascontiguousarray(x.T.astype(NPBF16))          # [D, N]
    scale_q = 1.0 / math.sqrt(HD)
    Wq_s = np.ascontiguousarray((np.asarray(Wq) * scale_q).astype(NPBF16))
    Wk_s = np.ascontiguousarray(np.asarray(Wk).astype(NPBF16))
    Wv_s = np.ascontiguousarray(np.asarray(Wv).astype(NPBF16))
    bias3 = np.zeros((128, 3, D), np.float32)
    bias3[:, 0, :] = np.asarray(bq) * scale_q
    bias3[:, 1, :] = bk
    bias3[:, 2, :] = bv
    Wo_bf = np.ascontiguousarray(np.asarray(Wo).astype(NPBF16))

    iota = np.ascontiguousarray(
        np.broadcast_to(np.arange(128, dtype=np.float32), (128, 128))
    ).astype(NPBF16)
    ident = np.eye(128, dtype=np.float32).astype(NPBF16)
    g_rep = np.ascontiguousarray(
        np.broadcast_to(np.asarray(ln_g, np.float32), (128, D)))
    b_rep = np.ascontiguousarray(
        np.broadcast_to(np.asarray(ln_b, np.float32), (128, D)))

    in_maps = []
    for c in range(NCORES):
        rows = slice(c * NR, (c + 1) * NR)
        xTown = np.ascontiguousarray(xT[:, rows])
        xb = np.ascontiguousarray(x[rows] + np.asarray(bo, np.float32))
        m = dict(
            xTown=xTown, Wq=Wq_s, Wk=Wk_s, Wv=Wv_s,
            bias3=bias3, Wo=Wo_bf, xb=xb, g_rep=g_rep, b_rep=b_rep,
            iota=iota, ident=ident,
            **edges[c],
        )
        in_maps.append(m)
    zero_bias = not (np.any(bias3) or False)
    meta = dict(C_BLK=C_BLK, NCHUNK=NCHUNK, EP=EP, zero_bias=zero_bias)
    return in_maps, meta


def build_program(cfg, C_BLK, zero_bias=False):
    N, D, H, HD = cfg["N"], cfg["D"], cfg["H"], cfg["HD"]
    NR, NBLK, DC = cfg["NR"], cfg["NBLK"], cfg["DC"]
    NCHUNK = NBLK * C_BLK
    EP = NCHUNK * 128
    NT = N // 128
    NTO = NR // 128

    nc = bacc.Bacc(None, target_bir_lowering=False, debug=False,
                   num_devices=NCORES)

    xTown_in = nc.dram_tensor("xTown", [D, NR], BF16, kind="ExternalInput")
    Wq_in = nc.dram_tensor("Wq", [D, D], BF16, kind="ExternalInput")
    Wk_in = nc.dram_tensor("Wk", [D, D], BF16, kind="ExternalInput")
    Wv_in = nc.dram_tensor("Wv", [D, D], BF16, kind="ExternalInput")
    bias3_in = nc.dram_tensor("bias3", [128, 3, D], F32, kind="ExternalInput")
    Wo_in = nc.dram_tensor("Wo", [D, D], BF16, kind="ExternalInput")
    xb_in = nc.dram_tensor("xb", [NR, D], F32, kind="ExternalInput")
    g_rep_in = nc.dram_tensor("g_rep", [128, D], F32, kind="ExternalInput")
    b_rep_in = nc.dram_tensor("b_rep", [128, D], F32, kind="ExternalInput")
    iota_in = nc.dram_tensor("iota", [128, 128], BF16, kind="ExternalInput")
    ident_in = nc.dram_tensor("ident", [128, 128], BF16, kind="ExternalInput")
    src16_in = nc.dram_tensor("src16", [128, EP // 16], I16, kind="ExternalInput")
    dstloc_in = nc.dram_tensor("dstloc", [128, NCHUNK], F32, kind="ExternalInput")
    valid_in = nc.dram_tensor("valid", [128, NCHUNK], BF16, kind="ExternalInput")

    out_ext = nc.dram_tensor("out", [NR, D], F32, kind="ExternalOutput")

    AG = [list(range(NCORES))]

    with tile.TileContext(nc) as tc:
        with (
            tc.tile_pool(name="res", bufs=1) as res,
            tc.tile_pool(name="dram", bufs=1, space="DRAM") as dram,
        ):
            iota_sb = res.tile([128, 128], BF16)
            nc.sync.dma_start(iota_sb[:], iota_in[:])
            ident_sb = res.tile([128, 128], BF16)
            nc.sync.dma_start(ident_sb[:], ident_in[:])
            ln09_sb = res.tile([128, 1], F32)
            nc.vector.memset(ln09_sb[:], float(np.log(0.9)))
            src16_sb = res.tile([128, EP // 16], I16)
            nc.sync.dma_start(src16_sb[:], src16_in[:])
            dstloc_sb = res.tile([128, NCHUNK], F32)
            nc.sync.dma_start(dstloc_sb[:], dstloc_in[:])
            escale_sb = res.tile([128, NCHUNK, H, 2], BF16)
            v01_sb = res.tile([128, NTO, D], BF16)
            h5_sb = res.tile([128, NTO, D], BF16)
            q_sb = res.tile([128, NTO, D], BF16)
            rdenom_sb = res.tile([128, NBLK, H], F32)

            kv_own = dram.tile([NR, 2 * D], BF16)
            kv_dram = dram.tile([N, 2 * D], BF16, addr_space="Shared")
            # Shared DRAM is single-writer: one AllGather output per step.
            hs_dram = [dram.tile([N, D], BF16, addr_space="Shared",
                                 name=f"hstep{i}") for i in range(4)]
            shard = dram.tile([NR, D], BF16)

            # =========== P0: projections ===========
            with (
                tc.tile_pool(name="p0", bufs=3) as p0,
                tc.tile_pool(name="p0c", bufs=1) as p0c,
                tc.tile_pool(name="p0ps", bufs=4, space="PSUM") as p0ps,
            ):
                xTo_sb = p0c.tile([128, DC, NR], BF16)
                nc.sync.dma_start(
                    xTo_sb[:], xTown_in[:].rearrange("(c p) n -> p c n", p=128))
                W_sb = p0c.tile([128, 3, DC, D], BF16)
                for i, W in enumerate([Wq_in, Wk_in, Wv_in]):
                    nc.sync.dma_start(
                        W_sb[:, i, :, :],
                        W[:].rearrange("(c p) g -> p c g", p=128))
                bias_sb = p0c.tile([128, 3, D], F32)
                nc.sync.dma_start(bias_sb[:], bias3_in[:])

                def proj_tile(xsrc, toff, wi, stores, v01_t=None, out_ap=None):
                    ps = p0ps.tile([128, D], F32, tag="pps")
                    for c in range(DC):
                        for j in range(2):
                            js = slice(j * 512, min((j + 1) * 512, D))
                            nc.tensor.matmul(
                                ps[:, js], xsrc[:, c, toff:toff + 128],
                                W_sb[:, wi, c, js],
                                start=(c == 0), stop=(c == DC - 1))
                    if out_ap is None:
                        stg = p0.tile([128, D], BF16, tag="pstg")
                        stg_ap = stg[:]
                    else:
                        stg_ap = out_ap
                    if zero_bias:
                        nc.scalar.copy(stg_ap, ps[:])
                    else:
                        nc.vector.tensor_tensor(
                            stg_ap, ps[:], bias_sb[:, wi, :],
                            mybir.AluOpType.add)
                    for dest in stores:
                        nc.sync.dma_start(dest, stg_ap)
                    if v01_t is not None:
                        nc.vector.tensor_scalar(
                            v01_sb[:, v01_t, :], stg_ap, ALPHA, None,
                            mybir.AluOpType.mult)

                # own-shard k, v -> kv_own, AllGather into the global kv
                # gather table; q stays in SBUF for the PE one-hot expansion.
                for t in range(NTO):
                    r = slice(t * 128, (t + 1) * 128)
                    proj_tile(xTo_sb, t * 128, 1, [kv_own[r, 0:D]])
                for t in range(NTO):
                    r = slice(t * 128, (t + 1) * 128)
                    proj_tile(xTo_sb, t * 128, 2, [kv_own[r, D:2 * D]],
                              v01_t=t)
                nc.gpsimd.collective_compute(
                    "AllGather", mybir.AluOpType.bypass,
                    replica_groups=AG,
                    ins=[kv_own[:].opt()], outs=[kv_dram[:].opt()])
                for t in range(NTO):
                    proj_tile(xTo_sb, t * 128, 0, [], out_ap=q_sb[:, t, :])

            # =========== P1+P2: scores (step 0) + diffusion ===========
            with (
                tc.tile_pool(name="p2kv", bufs=2) as p2kv,
                tc.tile_pool(name="p2h", bufs=3) as p2h,
                tc.tile_pool(name="p2o", bufs=6) as p2o,
                tc.tile_pool(name="p2s", bufs=2) as p2s,
                tc.tile_pool(name="p2e", bufs=1) as p2e,
                tc.tile_pool(name="p2ps", bufs=2, space="PSUM") as p2ps,
                tc.tile_pool(name="p2qe", bufs=1, space="PSUM") as p2qe,
                tc.tile_pool(name="p2tp", bufs=1, space="PSUM") as p2tp,
            ):
                valid_sb = p2e.tile([128, NCHUNK], BF16)
                nc.sync.dma_start(valid_sb[:], valid_in[:])

                hsrcs = [kv_dram] + hs_dram
                hdsts = hs_dram + [None]
                for s in range(NSTEPS):
                    G = None
                    vG = None
                    psm = psd = None
                    oh = None
                    for q in range(NCHUNK):
                        blk, ch = divmod(q, C_BLK)
                        gc = q % GCH
                        if gc == 0:
                            ic = q * 8
                            gs = slice(q, q + GCH)
                            if s == 0:
                                G = p2kv.tile([128, GCH, 2 * D], BF16,
                                              tag="Gkv")
                                nc.gpsimd.dma_gather(
                                    G[:], kv_dram[:],
                                    src16_sb[:, ic:ic + GCH * 8],
                                    GCH * 128, GCH * 128, 2 * D)
                                vG = G[:, :, D:2 * D]
                            else:
                                G = p2h.tile([128, GCH, D], BF16, tag="Gh")
                                nc.gpsimd.dma_gather(
                                    G[:], hsrcs[s][:],
                                    src16_sb[:, ic:ic + GCH * 8],
                                    GCH * 128, GCH * 128, D)
                                vG = G[:, :, :]
                            oh = p2o.tile([128, GCH, 128], BF16, tag="oh")
                            nc.vector.tensor_tensor(
                                oh[:],
                                iota_sb[:].unsqueeze(1).to_broadcast(
                                    (128, GCH, 128)),
                                dstloc_sb[:, gs].unsqueeze(2).to_broadcast(
                                    (128, GCH, 128)),
                                mybir.AluOpType.is_equal)
                            if s == 0:
                                # scores: expand q[dst] per edge on PE via
                                # the transposed dst one-hot, then dot with
                                # the gathered k rows on DVE.
                                for g2 in range(GCH):
                                    qq = q + g2
                                    bq2 = qq // C_BLK
                                    tp = p2tp.tile([128, 128], BF16,
                                                   tag="tp")
                                    nc.tensor.transpose(
                                        tp[:], oh[:, g2, :], ident_sb[:])
                                    ohT = p2s.tile([128, 128], BF16,
                                                   tag="ohT")
                                    nc.scalar.copy(ohT[:], tp[:])
                                    qe = p2qe.tile([128, D], F32, tag="qe")
                                    for j in range(2):
                                        js = slice(j * 512,
                                                   min((j + 1) * 512, D))
                                        nc.tensor.matmul(
                                            qe[:, js], ohT[:],
                                            q_sb[:, bq2, js],
                                            start=True, stop=True)
                                    kq = p2s.tile([128, D], BF16, tag="kq")
                                    nc.vector.tensor_tensor(
                                        kq[:], G[:, g2, 0:D], qe[:],
                                        mybir.AluOpType.mult)
                                    sc = p2s.tile([128, H], F32, tag="sc")
                                    nc.vector.tensor_reduce(
                                        sc[:],
                                        kq[:].rearrange(
                                            "p (h f) -> p h f", h=H),
                                        mybir.AxisListType.X,
                                        mybir.AluOpType.add)
                                    nc.scalar.activation(
                                        escale_sb[:, qq, :, :],
                                        sc[:].unsqueeze(2).to_broadcast(
                                            (128, H, 2)),
                                        mybir.ActivationFunctionType.Exp,
                                        bias=ln09_sb[:], scale=1.0)
                                nc.vector.tensor_tensor(
                                    escale_sb[:, gs, :, :],
                                    escale_sb[:, gs, :, :],
                                    valid_sb[:, gs].unsqueeze(2)
                                    .to_broadcast((128, GCH, H))
                                    .unsqueeze(3)
                                    .to_broadcast((128, GCH, H, 2)),
                                    mybir.AluOpType.mult)
                            nc.vector.tensor_tensor(
                                vG.rearrange(
                                    "p c (h f2 two) -> p c h f2 two",
                                    h=H, two=2),
                                vG.rearrange(
                                    "p c (h f2 two) -> p c h f2 two",
                                    h=H, two=2),
                                escale_sb[:, gs, :, :].unsqueeze(3)
                                .to_broadcast(
                                    (128, GCH, H, HD // 2, 2)),
                                mybir.AluOpType.mult)
                        if ch == 0:
                            # psd (softmax denom) lives in the spare columns
                            # [D:D+H] of the psm tile - same 2 PSUM banks.
                            psm = p2ps.tile([128, D + H], F32, tag="psm")
                            psd = psm[:, D:D + H]
                        voff = D if s == 0 else 0
                        for j in range(2):
                            js = slice(voff + j * 512,
                                       voff + min((j + 1) * 512, D))
                            os_ = slice(j * 512, min((j + 1) * 512, D))
                            nc.tensor.matmul(
                                psm[:, os_], oh[:, gc, :], G[:, gc, js],
                                start=(ch == 0), stop=(ch == C_BLK - 1))
                        if s == 0:
                            nc.tensor.matmul(
                                psd, oh[:, gc, :],
                                escale_sb[:, q, :, 0:1].rearrange(
                                    "p h one -> p (h one)"),
                                start=(ch == 0), stop=(ch == C_BLK - 1))
                        if ch == C_BLK - 1:
                            if s == 0:
                                dn = p2s.tile([128, H], F32, tag="dn")
                                nc.vector.tensor_scalar(
                                    dn[:], psd, 0.9e-9, None,
                                    mybir.AluOpType.max)
                                dn2 = p2s.tile([128, H], F32, tag="dn2")
                                nc.vector.reciprocal(dn2[:], dn[:])
                                nc.vector.tensor_scalar(
                                    rdenom_sb[:, blk, :], dn2[:], 0.9, None,
                                    mybir.AluOpType.mult)
                            if s == NSTEPS - 1:
                                stg_ap = h5_sb[:, blk, :]
                            else:
                                stg = p2s.tile([128, D], BF16, tag="hstg")
                                stg_ap = stg[:]
                            for h in range(H):
                                hs = slice(h * HD, (h + 1) * HD)
                                nc.vector.scalar_tensor_tensor(
                                    stg_ap[:, hs], psm[:, hs],
                                    rdenom_sb[:, blk, h:h + 1],
                                    v01_sb[:, blk, hs],
                                    mybir.AluOpType.mult, mybir.AluOpType.add)
                            if s < NSTEPS - 1:
                                nc.sync.dma_start(
                                    shard[blk * 128:(blk + 1) * 128, :],
                                    stg_ap)
                    if s < NSTEPS - 1:
                        nc.gpsimd.collective_compute(
                            "AllGather", mybir.AluOpType.bypass,
                            replica_groups=AG,
                            ins=[shard[:].opt()], outs=[hdsts[s][:].opt()])

            # =========== P3: output projection + LN ===========
            with (
                tc.tile_pool(name="p3", bufs=2) as p3,
                tc.tile_pool(name="p3c", bufs=1) as p3c,
                tc.tile_pool(name="p3ps", bufs=4, space="PSUM") as p3ps,
                tc.tile_pool(name="p3ps2", bufs=2, space="PSUM") as p3ps2,
            ):
                g_sb = p3c.tile([128, D], F32)
                nc.sync.dma_start(g_sb[:], g_rep_in[:])
                b_sb = p3c.tile([128, D], F32)
                nc.sync.dma_start(b_sb[:], b_rep_in[:])
                h5T_sb = p3c.tile([128, DC, NR], BF16)
                for t in range(NTO):
                    for c in range(DC):
                        tp = p3ps.tile([128, 128], BF16, tag="tp")
                        nc.tensor.transpose(
                            tp[:], h5_sb[:, t, c * 128:(c + 1) * 128],
                            ident_sb[:])
                        nc.vector.tensor_copy(
                            h5T_sb[:, c, t * 128:(t + 1) * 128], tp[:])
                Wo_sb = p3c.tile([128, DC, D], BF16)
                nc.sync.dma_start(
                    Wo_sb[:], Wo_in[:].rearrange("(c p) n -> p c n", p=128))
                for t in range(NTO):
                    yps = p3ps2.tile([128, D], F32, tag="yps")
                    for c in range(DC):
                        for j in range(2):
                            js = slice(j * 512, min((j + 1) * 512, D))
                            nc.tensor.matmul(
                                yps[:, js],
                                h5T_sb[:, c, t * 128:(t + 1) * 128],
                                Wo_sb[:, c, js],
                                start=(c == 0), stop=(c == DC - 1))
                    xb_sb = p3.tile([128, D], F32, tag="xb")
                    nc.sync.dma_start(xb_sb[:], xb_in[t * 128:(t + 1) * 128, :])
                    y_sb = p3.tile([128, D], F32, tag="y")
                    nc.vector.tensor_tensor(
                        y_sb[:], yps[:], xb_sb[:], mybir.AluOpType.add)
                    mu = p3.tile([128, 1], F32, tag="mu")
                    nc.vector.tensor_reduce(
                        mu[:], y_sb[:], mybir.AxisListType.X,
                        mybir.AluOpType.add)
                    negmu = p3.tile([128, 1], F32, tag="negmu")
                    nc.vector.tensor_scalar(
                        negmu[:], mu[:], -1.0 / D, None, mybir.AluOpType.mult)
                    sq = p3.tile([128, D], F32, tag="sq")
                    var = p3.tile([128, 1], F32, tag="var")
                    nc.scalar.activation(
                        sq[:], y_sb[:], mybir.ActivationFunctionType.Square,
                        bias=negmu[:], scale=1.0, accum_out=var[:])
                    vs = p3.tile([128, 1], F32, tag="vs")
                    nc.vector.tensor_scalar(
                        vs[:], var[:], 1.0 / D, LN_EPS,
                        mybir.AluOpType.mult, mybir.AluOpType.add)
                    std = p3.tile([128, 1], F32, tag="std")
                    nc.scalar.sqrt(std[:], vs[:])
                    rstd = p3.tile([128, 1], F32, tag="rstd")
                    nc.vector.reciprocal(rstd[:], std[:])
                    t1 = p3.tile([128, D], F32, tag="t1")
                    nc.vector.scalar_tensor_tensor(
                        t1[:], y_sb[:], negmu[:], g_sb[:],
                        mybir.AluOpType.add, mybir.AluOpType.mult)
                    outt = p3.tile([128, D], F32, tag="outt")
                    nc.vector.scalar_tensor_tensor(
                        outt[:], t1[:], rstd[:], b_sb[:],
                        mybir.AluOpType.mult, mybir.AluOpType.add)
                    nc.sync.dma_start(
                        out_ext[t * 128:(t + 1) * 128, :], outt[:])

    nc.compile()
    return nc


_PROG_CACHE = {}


def _get_program(cfg, C_BLK, zero_bias):
    key = (cfg["N"], cfg["E"], cfg["D"], cfg["H"], C_BLK, zero_bias)
    if key not in _PROG_CACHE:
        _PROG_CACHE[key] = build_program(cfg, C_BLK, zero_bias)
    return _PROG_CACHE[key]


def run(cfg, inputs, trace=False):
    in_maps, meta = host_prep(cfg, **inputs)
    nc = _get_program(cfg, meta["C_BLK"], meta["zero_bias"])
    res = run_bass_kernel_spmd(
        nc, in_maps, core_ids=list(range(NCORES)), trace=trace)
    N, D, NR = cfg["N"], cfg["D"], cfg["NR"]
    full = np.empty((N, D), np.float32)
    for r in range(NRANGE):
        full[r * NR:(r + 1) * NR] = res.results[r]["out"]
    return full.reshape(cfg["B"], cfg["S"], D), res


def kernel(**inputs):
    cfg = _cfg(B=2, S=4096, D=768, H=12, E=524288)
    out, _ = run(cfg, inputs)
    return out



# revision 30
# speedup vs baseline: 1.2152x; 1.2152x over previous
"""Trainium2 Bass kernel for DiffuserAttention (GNN message passing).

v2 sharding: 8-way by dst-node range (1024 dst rows per core, full 768
feature width).  Edge scores are folded into diffusion step 1 via a
concatenated [k | v] gather table, so per-edge k rows ride the step-1
message gather for free.  Edge softmax is unnormalized (escale =
0.9*exp(score)); normalization (1/denom) is applied per-dst after each
segment-sum.  Per step: dma_gather h[src] rows (1536B/edge), DVE
broadcast-scale by escale, PE one-hot segment-sum into PSUM, per-dst
rdenom scale + 0.1*v teleport, 8-rank AllGather of the h shard (steps
1-4 only; step 5 output stays local for the output projection +
residual + LayerNorm).
"""

import sys

sys.path.insert(0, "/opt/trn_rl_repo")

import math

import numpy as np
import ml_dtypes

import concourse.bass as bass
import concourse.bacc as bacc
import concourse.mybir as mybir
import concourse.tile as tile
from concourse.bass_utils import run_bass_kernel_spmd

F32 = mybir.dt.float32
BF16 = mybir.dt.bfloat16
FP8 = mybir.dt.float8e4
I16 = mybir.dt.int16
NPBF16 = ml_dtypes.bfloat16

NCORES = 8
NRANGE = 8
ALPHA = 0.1
NSTEPS = 5
LN_EPS = 1e-12
GCH = 8      # chunks per gather group (1024 idx = SWDGE ring capacity)


def _cfg(B, S, D, H, E):
    N = B * S
    cfg = dict(
        B=B, S=S, D=D, H=H, E=E, N=N,
        HD=D // H,
        NR=N // NRANGE,
    )
    cfg["NBLK"] = cfg["NR"] // 128
    cfg["DC"] = D // 128
    return cfg


def wrap_idx(idx):
    """dma_gather index layout: [128, n/16] int16; idx i at [i%16, i//16],
    replicated across the 8 Q7 cores."""
    n = idx.shape[0]
    w = idx.reshape(n // 16, 16).T.astype(np.int16)
    return np.ascontiguousarray(np.tile(w, (8, 1)))


def host_prep(cfg, hidden_states, attention_mask, src, dst,
              Wq, bq, Wk, bk, Wv, bv, Wo, bo, ln_g, ln_b):
    N, D, H, HD = cfg["N"], cfg["D"], cfg["H"], cfg["HD"]
    NR, NBLK = cfg["NR"], cfg["NBLK"]

    x = np.asarray(hidden_states, np.float32).reshape(N, D)
    src = np.asarray(src).astype(np.int64)
    dst = np.asarray(dst).astype(np.int64)
    mask1 = np.asarray(attention_mask).reshape(-1) >= 0
    all_valid = bool(mask1.all())

    # ---- edge partition by dst range, sort by dst, pad per 128-dst block
    per_range = []
    maxchunks = 0
    for r in range(NRANGE):
        sel = np.nonzero((dst >= r * NR) & (dst < (r + 1) * NR))[0]
        dl = dst[sel] - r * NR
        order = np.argsort(dl, kind="stable")
        sel = sel[order]
        dl = dl[order]
        counts = np.bincount(dl >> 7, minlength=NBLK)
        maxchunks = max(maxchunks, int(np.ceil(counts / 128).max()))
        per_range.append((sel, dl, counts))

    C_BLK = maxchunks
    while (NBLK * C_BLK) % GCH:
        C_BLK += 1
    NCHUNK = NBLK * C_BLK
    EP = NCHUNK * 128

    edges = []
    for r in range(NRANGE):
        sel, dl, counts = per_range[r]
        src_e = np.zeros(EP, np.int16)
        dstq_e = np.zeros(EP, np.int16)
        dstloc_e = np.zeros(EP, np.float32)
        valid_e = np.zeros(EP, np.float32)
        starts = np.concatenate([[0], np.cumsum(counts)])
        for b in range(NBLK):
            s0, s1 = starts[b], starts[b + 1]
            n = s1 - s0
            o = b * C_BLK * 128
            src_e[o:o + n] = src[sel[s0:s1]]
            dstq_e[o:o + n] = dl[s0:s1]
            dstloc_e[o:o + n] = (dl[s0:s1] - b * 128).astype(np.float32)
            if all_valid:
                valid_e[o:o + n] = 1.0
            else:
                valid_e[o:o + n] = (
                    mask1[src[sel[s0:s1]]] & mask1[dst[sel[s0:s1]]]
                ).astype(np.float32)
        edges.append(dict(
            src16=wrap_idx(src_e),
            dstq16=wrap_idx(dstq_e),
            dstloc=np.ascontiguousarray(dstloc_e.reshape(NCHUNK, 128).T),
            valid=np.ascontiguousarray(
                valid_e.reshape(NCHUNK, 128).T.astype(NPBF16)),
        ))

    # ---- weights / constants
    xT = np.ascontiguousarray(x.T.astype(NPBF16))          # [D, N]
    scale_q = 1.0 / math.sqrt(HD)
    Wq_s = np.ascontiguousarray((np.asarray(Wq) * scale_q).astype(NPBF16))
    Wk_s = np.ascontiguousarray(np.asarray(Wk).astype(NPBF16))
    Wv_s = np.ascontiguousarray(np.asarray(Wv).astype(NPBF16))
    bias3 = np.zeros((128, 3, D), np.float32)
    bias3[:, 0, :] = np.asarray(bq) * scale_q
    bias3[:, 1, :] = bk
    bias3[:, 2, :] = bv
    Wo_bf = np.ascontiguousarray(np.asarray(Wo).astype(NPBF16))

    iota = np.ascontiguousarray(
        np.broadcast_to(np.arange(128, dtype=np.float32), (128, 128))
    ).astype(NPBF16)
    ident = np.eye(128, dtype=np.float32).astype(NPBF16)
    g_rep = np.ascontiguousarray(
        np.broadcast_to(np.asarray(ln_g, np.float32), (128, D)))
    b_rep = np.ascontiguousarray(
        np.broadcast_to(np.asarray(ln_b, np.float32), (128, D)))

    in_maps = []
    for c in range(NCORES):
        rows = slice(c * NR, (c + 1) * NR)
        xTown = np.ascontiguousarray(xT[:, rows])
        xb = np.ascontiguousarray(x[rows] + np.asarray(bo, np.float32))
        m = dict(
            xTown=xTown, Wq=Wq_s, Wk=Wk_s, Wv=Wv_s,
            bias3=bias3, Wo=Wo_bf, xb=xb, g_rep=g_rep, b_rep=b_rep,
            iota=iota, ident=ident,
            **edges[c],
        )
        in_maps.append(m)
    zero_bias = not (np.any(bias3) or False)
    meta = dict(C_BLK=C_BLK, NCHUNK=NCHUNK, EP=EP, zero_bias=zero_bias)
    return in_maps, meta


def build_program(cfg, C_BLK, zero_bias=False):
    N, D, H, HD = cfg["N"], cfg["D"], cfg["H"], cfg["HD"]
    NR, NBLK, DC = cfg["NR"], cfg["NBLK"], cfg["DC"]
    NCHUNK = NBLK * C_BLK
    EP = NCHUNK * 128
    NT = N // 128
    NTO = NR // 128

    nc = bacc.Bacc(None, target_bir_lowering=False, debug=False,
                   num_devices=NCORES)

    xTown_in = nc.dram_tensor("xTown", [D, NR], BF16, kind="ExternalInput")
    Wq_in = nc.dram_tensor("Wq", [D, D], BF16, kind="ExternalInput")
    Wk_in = nc.dram_tensor("Wk", [D, D], BF16, kind="ExternalInput")
    Wv_in = nc.dram_tensor("Wv", [D, D], BF16, kind="ExternalInput")
    bias3_in = nc.dram_tensor("bias3", [128, 3, D], F32, kind="ExternalInput")
    Wo_in = nc.dram_tensor("Wo", [D, D], BF16, kind="ExternalInput")
    xb_in = nc.dram_tensor("xb", [NR, D], F32, kind="ExternalInput")
    g_rep_in = nc.dram_tensor("g_rep", [128, D], F32, kind="ExternalInput")
    b_rep_in = nc.dram_tensor("b_rep", [128, D], F32, kind="ExternalInput")
    iota_in = nc.dram_tensor("iota", [128, 128], BF16, kind="ExternalInput")
    ident_in = nc.dram_tensor("ident", [128, 128], BF16, kind="ExternalInput")
    src16_in = nc.dram_tensor("src16", [128, EP // 16], I16, kind="ExternalInput")
    dstq16_in = nc.dram_tensor("dstq16", [128, EP // 16], I16, kind="ExternalInput")
    dstloc_in = nc.dram_tensor("dstloc", [128, NCHUNK], F32, kind="ExternalInput")
    valid_in = nc.dram_tensor("valid", [128, NCHUNK], BF16, kind="ExternalInput")

    out_ext = nc.dram_tensor("out", [NR, D], F32, kind="ExternalOutput")

    AG = [list(range(NCORES))]

    with tile.TileContext(nc) as tc:
        with (
            tc.tile_pool(name="res", bufs=1) as res,
            tc.tile_pool(name="dram", bufs=1, space="DRAM") as dram,
        ):
            iota_sb = res.tile([128, 128], BF16)
            nc.sync.dma_start(iota_sb[:], iota_in[:])
            ident_sb = res.tile([128, 128], BF16)
            nc.sync.dma_start(ident_sb[:], ident_in[:])
            ln09_sb = res.tile([128, 1], F32)
            nc.vector.memset(ln09_sb[:], float(np.log(0.9)))
            src16_sb = res.tile([128, EP // 16], I16)
            nc.sync.dma_start(src16_sb[:], src16_in[:])
            dstq16_sb = res.tile([128, EP // 16], I16)
            nc.sync.dma_start(dstq16_sb[:], dstq16_in[:])
            dstloc_sb = res.tile([128, NCHUNK], F32)
            nc.sync.dma_start(dstloc_sb[:], dstloc_in[:])
            escale_sb = res.tile([128, NCHUNK, H, 2], BF16)
            v01_sb = res.tile([128, NTO, D], BF16)
            h5_sb = res.tile([128, NTO, D], BF16)
            q_sb = res.tile([128, NTO, D], BF16)
            rdenom_sb = res.tile([128, NBLK, H], F32)

            # fp8 payload tables: halves gather/AllGather bytes. NOTE:
            # Shared addr_space makes the AllGather ~2x faster but slows
            # every dma_gather FROM the tensor by ~26% - net loss.
            kv_own = dram.tile([NR, 2 * D], FP8)
            kv_dram = dram.tile([N, 2 * D], FP8)
            q_dram = dram.tile([NR, D], FP8)
            hs_dram = [dram.tile([N, D], FP8, name=f"hstep{i}")
                       for i in range(4)]
            shard = dram.tile([NR, D], FP8)

            # =========== P0: projections ===========
            with (
                tc.tile_pool(name="p0", bufs=3) as p0,
                tc.tile_pool(name="p0c", bufs=1) as p0c,
                tc.tile_pool(name="p0ps", bufs=4, space="PSUM") as p0ps,
            ):
                xTo_sb = p0c.tile([128, DC, NR], BF16)
                nc.sync.dma_start(
                    xTo_sb[:], xTown_in[:].rearrange("(c p) n -> p c n", p=128))
                W_sb = p0c.tile([128, 3, DC, D], BF16)
                for i, W in enumerate([Wq_in, Wk_in, Wv_in]):
                    nc.sync.dma_start(
                        W_sb[:, i, :, :],
                        W[:].rearrange("(c p) g -> p c g", p=128))
                bias_sb = p0c.tile([128, 3, D], F32)
                nc.sync.dma_start(bias_sb[:], bias3_in[:])

                def proj_tile(xsrc, toff, wi, fp8_stores, v01_t=None,
                              out_ap=None):
                    ps = p0ps.tile([128, D], F32, tag="pps")
                    for c in range(DC):
                        for j in range(2):
                            js = slice(j * 512, min((j + 1) * 512, D))
                            nc.tensor.matmul(
                                ps[:, js], xsrc[:, c, toff:toff + 128],
                                W_sb[:, wi, c, js],
                                start=(c == 0), stop=(c == DC - 1))
                    if out_ap is None:
                        stg = p0.tile([128, D], BF16, tag="pstg")
                        stg_ap = stg[:]
                    else:
                        stg_ap = out_ap
                    if zero_bias:
                        nc.scalar.copy(stg_ap, ps[:])
                    else:
                        nc.vector.tensor_tensor(
                            stg_ap, ps[:], bias_sb[:, wi, :],
                            mybir.AluOpType.add)
                    if fp8_stores:
                        st8 = p0.tile([128, D], FP8, tag="st8")
                        nc.scalar.copy(st8[:], stg_ap)
                        for dest in fp8_stores:
                            nc.sync.dma_start(dest, st8[:])
                    if v01_t is not None:
                        # teleport term from the f32 accumulator (not the
                        # fp8-quantized store)
                        nc.vector.tensor_scalar(
                            v01_sb[:, v01_t, :], ps[:], ALPHA, None,
                            mybir.AluOpType.mult)

                # own-shard k, v -> kv_own, AllGather into the global kv
                # gather table; q both to SBUF (qexp path) and to q_dram
                # (gathered-q path).
                for t in range(NTO):
                    r = slice(t * 128, (t + 1) * 128)
                    proj_tile(xTo_sb, t * 128, 1, [kv_own[r, 0:D]])
                for t in range(NTO):
                    r = slice(t * 128, (t + 1) * 128)
                    proj_tile(xTo_sb, t * 128, 2, [kv_own[r, D:2 * D]],
                              v01_t=t)
                nc.gpsimd.collective_compute(
                    "AllGather", mybir.AluOpType.bypass,
                    replica_groups=AG,
                    ins=[kv_own[:].opt()], outs=[kv_dram[:].opt()])
                for t in range(NTO):
                    proj_tile(xTo_sb, t * 128, 0,
                              [q_dram[t * 128:(t + 1) * 128, :]],
                              out_ap=q_sb[:, t, :])

            # =========== P1+P2: scores (step 0) + diffusion ===========
            with (
                tc.tile_pool(name="p2kv", bufs=3) as p2kv,
                tc.tile_pool(name="p2h", bufs=3) as p2h,
                tc.tile_pool(name="p2q", bufs=2) as p2q,
                tc.tile_pool(name="p2w", bufs=2) as p2w,
                tc.tile_pool(name="p2o", bufs=6) as p2o,
                tc.tile_pool(name="p2s", bufs=2) as p2s,
                tc.tile_pool(name="p2e", bufs=1) as p2e,
                tc.tile_pool(name="p2ps", bufs=2, space="PSUM") as p2ps,
                tc.tile_pool(name="p2qe", bufs=1, space="PSUM") as p2qe,
                tc.tile_pool(name="p2tp", bufs=1, space="PSUM") as p2tp,
            ):
                valid_sb = p2e.tile([128, NCHUNK], BF16)
                nc.sync.dma_start(valid_sb[:], valid_in[:])

                hsrcs = [kv_dram] + hs_dram
                hdsts = hs_dram + [None]
                # chunks [0, NQE) of each block compute q[dst] via PE
                # one-hot expansion; the rest gather q_dram rows per edge.
                # Balances PE against the Q7 descriptor-generation cost.
                NQE = int(C_BLK * 0.36)
                for s in range(NSTEPS):
                    for blk in range(NBLK):
                        # psd (softmax denom) lives in the spare columns
                        # [D:D+H] of the psm tile - same 2 PSUM banks.
                        psm = p2ps.tile([128, D + H], F32, tag="psm")
                        psd = psm[:, D:D + H]
                        qcall = None
                        qcall_start = qcall_end = 0
                        for g0 in range(0, C_BLK, GCH):
                            g = min(GCH, C_BLK - g0)
                            q0 = blk * C_BLK + g0
                            ic = q0 * 8
                            gs = slice(q0, q0 + g)
                            if s == 0:
                                G = p2kv.tile([128, GCH, 2 * D], FP8,
                                              tag="Gkv")
                                nc.gpsimd.dma_gather(
                                    G[:, 0:g, :], kv_dram[:],
                                    src16_sb[:, ic:ic + g * 8],
                                    g * 128, g * 128, 2 * D)
                                vG = G[:, 0:g, D:2 * D]
                            else:
                                G = p2h.tile([128, GCH, D], FP8, tag="Gh")
                                nc.gpsimd.dma_gather(
                                    G[:, 0:g, :], hsrcs[s][:],
                                    src16_sb[:, ic:ic + g * 8],
                                    g * 128, g * 128, D)
                                vG = G[:, 0:g, :]
                            oh = p2o.tile([128, GCH, 128], BF16, tag="oh")
                            nc.vector.tensor_tensor(
                                oh[:, 0:g, :],
                                iota_sb[:].unsqueeze(1).to_broadcast(
                                    (128, g, 128)),
                                dstloc_sb[:, gs].unsqueeze(2).to_broadcast(
                                    (128, g, 128)),
                                mybir.AluOpType.is_equal)
                            if s == 0:
                                for gc in range(g):
                                    ch = g0 + gc
                                    qq = q0 + gc
                                    if ch < NQE:
                                        # PE path: qexp = ohT^T @ q_blk
                                        tp = p2tp.tile([128, 128], BF16,
                                                       tag="tp")
                                        nc.tensor.transpose(
                                            tp[:], oh[:, gc, :],
                                            ident_sb[:])
                                        ohT = p2s.tile([128, 128], BF16,
                                                       tag="ohT")
                                        nc.scalar.copy(ohT[:], tp[:])
                                        qe = p2qe.tile([128, D], F32,
                                                       tag="qe")
                                        for j in range(2):
                                            js = slice(j * 512,
                                                       min((j + 1) * 512,
                                                           D))
                                            nc.tensor.matmul(
                                                qe[:, js], ohT[:],
                                                q_sb[:, blk, js],
                                                start=True, stop=True)
                                        qsrc = qe[:]
                                    else:
                                        if qcall is None or ch >= qcall_end:
                                            qn = min(GCH, C_BLK - ch)
                                            qcall = p2q.tile(
                                                [128, GCH, D], FP8,
                                                tag="Qg")
                                            nc.gpsimd.dma_gather(
                                                qcall[:, 0:qn, :],
                                                q_dram[:],
                                                dstq16_sb[:, qq * 8:
                                                          (qq + qn) * 8],
                                                qn * 128, qn * 128, D)
                                            qcall_start = ch
                                            qcall_end = ch + qn
                                        qsrc = qcall[:, ch - qcall_start, :]
                                    kq = p2s.tile([128, D], BF16, tag="kq")
                                    nc.vector.tensor_tensor(
                                        kq[:], G[:, gc, 0:D], qsrc,
                                        mybir.AluOpType.mult)
                                    sc = p2s.tile([128, H], F32, tag="sc")
                                    nc.vector.tensor_reduce(
                                        sc[:],
                                        kq[:].rearrange(
                                            "p (h f) -> p h f", h=H),
                                        mybir.AxisListType.X,
                                        mybir.AluOpType.add)
                                    nc.scalar.activation(
                                        escale_sb[:, qq, :, :],
                                        sc[:].unsqueeze(2).to_broadcast(
                                            (128, H, 2)),
                                        mybir.ActivationFunctionType.Exp,
                                        bias=ln09_sb[:], scale=1.0)
                                nc.vector.tensor_tensor(
                                    escale_sb[:, gs, :, :],
                                    escale_sb[:, gs, :, :],
                                    valid_sb[:, gs].unsqueeze(2)
                                    .to_broadcast((128, g, H))
                                    .unsqueeze(3)
                                    .to_broadcast((128, g, H, 2)),
                                    mybir.AluOpType.mult)
                            vGw = p2w.tile([128, GCH, D], BF16, tag="vGw")
                            nc.vector.tensor_tensor(
                                vGw[:, 0:g, :].rearrange(
                                    "p c (h f2 two) -> p c h f2 two",
                                    h=H, two=2),
                                vG.rearrange(
                                    "p c (h f2 two) -> p c h f2 two",
                                    h=H, two=2),
                                escale_sb[:, gs, :, :].unsqueeze(3)
                                .to_broadcast(
                                    (128, g, H, HD // 2, 2)),
                                mybir.AluOpType.mult)
                            for gc in range(g):
                                ch = g0 + gc
                                for j in range(2):
                                    js = slice(j * 512,
                                               min((j + 1) * 512, D))
                                    nc.tensor.matmul(
                                        psm[:, js], oh[:, gc, :],
                                        vGw[:, gc, js],
                                        start=(ch == 0),
                                        stop=(ch == C_BLK - 1))
                                if s == 0:
                                    nc.tensor.matmul(
                                        psd, oh[:, gc, :],
                                        escale_sb[:, q0 + gc, :, 0:1]
                                        .rearrange("p h one -> p (h one)"),
                                        start=(ch == 0),
                                        stop=(ch == C_BLK - 1))
                        if s == 0:
                            dn = p2s.tile([128, H], F32, tag="dn")
                            nc.vector.tensor_scalar(
                                dn[:], psd, 0.9e-9, None,
                                mybir.AluOpType.max)
                            dn2 = p2s.tile([128, H], F32, tag="dn2")
                            nc.vector.reciprocal(dn2[:], dn[:])
                            nc.vector.tensor_scalar(
                                rdenom_sb[:, blk, :], dn2[:], 0.9, None,
                                mybir.AluOpType.mult)
                        if s == NSTEPS - 1:
                            stg_ap = h5_sb[:, blk, :]
                        else:
                            stg = p2s.tile([128, D], FP8, tag="hstg")
                            stg_ap = stg[:]
                        for h in range(H):
                            hs = slice(h * HD, (h + 1) * HD)
                            nc.vector.scalar_tensor_tensor(
                                stg_ap[:, hs], psm[:, hs],
                                rdenom_sb[:, blk, h:h + 1],
                                v01_sb[:, blk, hs],
                                mybir.AluOpType.mult, mybir.AluOpType.add)
                        if s < NSTEPS - 1:
                            nc.sync.dma_start(
                                shard[blk * 128:(blk + 1) * 128, :],
                                stg_ap)
                    if s < NSTEPS - 1:
                        nc.gpsimd.collective_compute(
                            "AllGather", mybir.AluOpType.bypass,
                            replica_groups=AG,
                            ins=[shard[:].opt()], outs=[hdsts[s][:].opt()])

            # =========== P3: output projection + LN ===========
            with (
                tc.tile_pool(name="p3", bufs=2) as p3,
                tc.tile_pool(name="p3c", bufs=1) as p3c,
                tc.tile_pool(name="p3ps", bufs=4, space="PSUM") as p3ps,
                tc.tile_pool(name="p3ps2", bufs=2, space="PSUM") as p3ps2,
            ):
                g_sb = p3c.tile([128, D], F32)
                nc.sync.dma_start(g_sb[:], g_rep_in[:])
                b_sb = p3c.tile([128, D], F32)
                nc.sync.dma_start(b_sb[:], b_rep_in[:])
                h5T_sb = p3c.tile([128, DC, NR], BF16)
                for t in range(NTO):
                    for c in range(DC):
                        tp = p3ps.tile([128, 128], BF16, tag="tp")
                        nc.tensor.transpose(
                            tp[:], h5_sb[:, t, c * 128:(c + 1) * 128],
                            ident_sb[:])
                        nc.vector.tensor_copy(
                            h5T_sb[:, c, t * 128:(t + 1) * 128], tp[:])
                Wo_sb = p3c.tile([128, DC, D], BF16)
                nc.sync.dma_start(
                    Wo_sb[:], Wo_in[:].rearrange("(c p) n -> p c n", p=128))
                for t in range(NTO):
                    yps = p3ps2.tile([128, D], F32, tag="yps")
                    for c in range(DC):
                        for j in range(2):
                            js = slice(j * 512, min((j + 1) * 512, D))
                            nc.tensor.matmul(
                                yps[:, js],
                                h5T_sb[:, c, t * 128:(t + 1) * 128],
                                Wo_sb[:, c, js],
                                start=(c == 0), stop=(c == DC - 1))
                    xb_sb = p3.tile([128, D], F32, tag="xb")
                    nc.sync.dma_start(xb_sb[:], xb_in[t * 128:(t + 1) * 128, :])
                    y_sb = p3.tile([128, D], F32, tag="y")
                    nc.vector.tensor_tensor(
                        y_sb[:], yps[:], xb_sb[:], mybir.AluOpType.add)
                    mu = p3.tile([128, 1], F32, tag="mu")
                    nc.vector.tensor_reduce(
                        mu[:], y_sb[:], mybir.AxisListType.X,
                        mybir.AluOpType.add)
                    negmu = p3.tile([128, 1], F32, tag="negmu")
                    nc.vector.tensor_scalar(
                        negmu[:], mu[:], -1.0 / D, None, mybir.AluOpType.mult)
                    sq = p3.tile([128, D], F32, tag="sq")
                    var = p3.tile([128, 1], F32, tag="var")
                    nc.scalar.activation(
                        sq[:], y_sb[:], mybir.ActivationFunctionType.Square,
                        bias=negmu[:], scale=1.0, accum_out=var[:])
                    vs = p3.tile([128, 1], F32, tag="vs")
                    nc.vector.tensor_scalar(
                        vs[:], var[:], 1.0 / D, LN_EPS,
                        mybir.AluOpType.mult, mybir.AluOpType.add)
                    std = p3.tile([128, 1], F32, tag="std")
                    nc.scalar.sqrt(std[:], vs[:])
                    rstd = p3.tile([128, 1], F32, tag="rstd")
                    nc.vector.reciprocal(rstd[:], std[:])
                    t1 = p3.tile([128, D], F32, tag="t1")
                    nc.vector.scalar_tensor_tensor(
                        t1[:], y_sb[:], negmu[:], g_sb[:],
                        mybir.AluOpType.add, mybir.AluOpType.mult)
                    outt = p3.tile([128, D], F32, tag="outt")
                    nc.vector.scalar_tensor_tensor(
                        outt[:], t1[:], rstd[:], b_sb[:],
                        mybir.AluOpType.mult, mybir.AluOpType.add)
                    nc.sync.dma_start(
                        out_ext[t * 128:(t + 1) * 128, :], outt[:])

    nc.compile()
    return nc


_PROG_CACHE = {}


def _get_program(cfg, C_BLK, zero_bias):
    key = (cfg["N"], cfg["E"], cfg["D"], cfg["H"], C_BLK, zero_bias)
    if key not in _PROG_CACHE:
        _PROG_CACHE[key] = build_program(cfg, C_BLK, zero_bias)
    return _PROG_CACHE[key]


def run(cfg, inputs, trace=False):
    in_maps, meta = host_prep(cfg, **inputs)
    nc = _get_program(cfg, meta["C_BLK"], meta["zero_bias"])
    res = run_bass_kernel_spmd(
        nc, in_maps, core_ids=list(range(NCORES)), trace=trace)
    N, D, NR = cfg["N"], cfg["D"], cfg["NR"]
    full = np.empty((N, D), np.float32)
    for r in range(NRANGE):
        full[r * NR:(r + 1) * NR] = res.results[r]["out"]
    return full.reshape(cfg["B"], cfg["S"], D), res


def kernel(**inputs):
    cfg = _cfg(B=2, S=4096, D=768, H=12, E=524288)
    out, _ = run(cfg, inputs)
    return out

